# revision 10
# baseline (speedup 1.0000x reference)
"""GQA attention (S=2048, D=2048, 32 q-heads / 8 kv-heads, rope, causal) on 8
Trainium2 NeuronCores, tensor-parallel over heads (1 kv head + 4 q heads per
core), chunked AllToAll re-shard overlapped with compute, row-sharded output.

v4 on top of v3 (trace-driven):
 - cos/sin/mask load as single contiguous DMAs (the chunked mid-axis slices
   fragmented into 640B descriptors that jammed the rings and stalled the
   PE ~40us waiting for x chunks queued behind them).
 - warm-up collective staged first on the sync ring -> cores barrier-align
   ~16us in instead of ~66us.
 - BOTH head-pairs' attention tiles t=0..2 interleave into stage P (kT/v
   are shared by the pairs); only the two t=3 tiles remain after block 15,
   so a2a(0) fires ~8us after stage P and a2a(1) ~8us later, with pair-1
   t3 and stage-W i=0 overlapping the collectives.
 - a2a staging DMAs issue incrementally as each tile is normalized.

v3 design vs the original baseline (408us):
 - stage P computes projections UNTRANSPOSED (seq on partitions, qkv dims on
   the free axis, one fused (128,384) q|k|v psum tile per 128-seq block).
   Rope pairs are then adjacent on the free axis, so the rotation runs as
   full-width (128,160) stride-2 vector ops instead of 30 (32,512)-shaped
   ops per seq tile. v lands directly in PV-lhs orientation; q/k are
   transposed by the PE via identity matmuls.
 - stage P is DMA-floor-paced (x streams ~9MB over 3 rings), so the first
   head-pair's ATTENTION IS INTERLEAVED INTO stage P: attention q-tile t
   only needs seq blocks 0..4t+3, so it runs right after P block 4t+3,
   filling the PE/ACT/DVE idle time under the DMA wait and firing the first
   AllToAll ~80us earlier. This also keeps the PE continuously busy -- its
   clock is activity-gated (HAM throttles to 1.2GHz when idling).
 - scores matmuls for the two heads of a pair run CONCURRENTLY in the PE
   array via row tiling (head A in rows 0:63 / tile_position (0,0), head B
   in rows 64:127 / (64,0)); kT is duplicated into partitions 64:128.
 - exp: diagonal tiles use exact ACT-table exp; full-width tiles are
   balanced between ACT and a paired-Schraudolph exp2 bit trick on the DVE
   (two f32->int16 tensor_scalar ops bitcast to bf16; the two PV matmuls
   against v and v/sqrt2 average the pair, cancelling most of the sawtooth
   error inside the softmax). Sub-B PV matmuls lag one block-step so the
   DVE latency never stalls the in-order PE.
 - PV matmuls use the full 128-col v tile (cols 65:128 zero) so fast weight
   load kicks in; the ones-column at col 64 yields softmax denominators.
 - stage W: i=1 pass is m-outer so the m=0 psum copy + output DMA overlap
   the m=1 matmuls.

Self-contained: takes full inputs, shards on host, runs one SPMD Bass/Tile
kernel via run_bass_kernel_spmd, reassembles the full output.
"""
import os
import sys
from contextlib import ExitStack

import numpy as np

try:
    import concourse.bass as bass  # noqa: F401
except ImportError:  # platform tree not on sys.path in a fresh dir
    sys.path.insert(0, "/opt/trn_rl_repo")
    import concourse.bass as bass  # noqa: F401

import concourse.mybir as mybir
from concourse import bacc, bass_utils, tile
from concourse.masks import make_identity

F32 = mybir.dt.float32
BF16 = mybir.dt.bfloat16
I16 = mybir.dt.int16
AF = mybir.ActivationFunctionType

S = 2048          # sequence length
D = 2048          # model dim
HD = 64           # head dim
N_CORES = 8
QH_PER_CORE = 4   # q heads per core (32/8)
QCOLS = QH_PER_CORE * HD       # 256 q-projection cols per core
PCOLS = QCOLS + 2 * HD         # 384 fused q|k|v cols per core
RCOLS = QCOLS + HD             # 320 roped cols (q|k)
ROWS_PER_CORE = S // N_CORES   # 256 output rows per core

SCH_A = 0.125 * 128 * 1.4426950408889634
# bias: 127*128 maps the exponent; -128 halves (the pair S1+S2/sqrt2 would
# otherwise sum to ~2*exp, mismatching exact-exp blocks in the same softmax
# chain); -6.87 removes the pair's +3.85% mean sawtooth bias so approx and
# exact blocks carry equal weight in the denominator.
SCH_B1 = 127.0 * 128.0 - 128.0 - 6.87
SCH_B2 = SCH_B1 + 64.0


def _build():
    nc = bacc.Bacc("TRN2", target_bir_lowering=False, debug=False,
                   num_devices=N_CORES)
    xT_d = nc.dram_tensor("xT", [2, 16, 128, 1024], BF16, kind="ExternalInput")
    wqkv_d = nc.dram_tensor("wqkv", [4, 128, 4, PCOLS], BF16,
                            kind="ExternalInput")
    wo_d = nc.dram_tensor("wo", [128, 16, D], BF16, kind="ExternalInput")
    cos5_d = nc.dram_tensor("cos5", [128, 16, RCOLS // 2], BF16,
                            kind="ExternalInput")
    sin5_d = nc.dram_tensor("sin5", [128, 16, RCOLS // 2], BF16,
                            kind="ExternalInput")
    mask_d = nc.dram_tensor("maskT01", [128, 128], BF16, kind="ExternalInput")
    out_d = nc.dram_tensor("out", [ROWS_PER_CORE, D], F32, kind="ExternalOutput")

    with tile.TileContext(nc) as tc, ExitStack() as top:
        persist = top.enter_context(tc.tile_pool(name="persist", bufs=1))
        # head-pair q tiles: rows = [head(2p) dims | head(2p+1) dims]
        qPs = [persist.tile([128, S], BF16, name=f"qP{i}", uniquify=False)
               for i in range(2)]
        # k dims on rows 0:64, duplicated to 64:128 for the paired scores mm
        kT = persist.tile([128, S], BF16, name="kT")
        v128 = persist.tile([128, 16, 128], BF16, name="v128")
        # v/sqrt2: second PV operand of the paired-Schraudolph approx path
        v128b = persist.tile([128, 16, 128], BF16, name="v128b")
        attnT0 = persist.tile([128, S], BF16, name="attnT0")
        attnT1 = persist.tile([128, S], BF16, name="attnT1")
        attnTs = [attnT0, attnT1]
        maskT_sb = persist.tile([128, 128], BF16, name="maskT_sb")
        cos5_sb = persist.tile([128, 16, RCOLS // 2], F32, name="cos5_sb")
        sin5_sb = persist.tile([128, 16, RCOLS // 2], F32, name="sin5_sb")
        wqkv_sb = persist.tile([128, 16, PCOLS], BF16, name="wqkv_sb")
        wo_sb = persist.tile([128, 16, D], BF16, name="wo_sb")
        identity = persist.tile([128, 128], F32, name="identity")
        wsmall = persist.tile([8, 64], BF16, name="wsmall")
        probs_pool = top.enter_context(tc.tile_pool(name="probs", bufs=3))
        nrm_pool = top.enter_context(tc.tile_pool(name="nrm", bufs=4))

        dram = top.enter_context(tc.tile_pool(name="dram", bufs=1, space="DRAM"))
        a2a_in = [dram.tile([N_CORES, 128, ROWS_PER_CORE], BF16,
                            name=f"a2a_in{i}", uniquify=False)
                  for i in range(2)]
        a2a_out = [dram.tile([N_CORES, 128, ROWS_PER_CORE], BF16,
                             name=f"a2a_out{i}", uniquify=False)
                   for i in range(2)]
        dum_in = dram.tile([N_CORES, 64], BF16, name="dum_in", uniquify=False)
        dum_out = dram.tile([N_CORES, 64], BF16, name="dum_out",
                            uniquify=False)

        # warm-up collective FIRST: its 1KB staging lands before the big
        # streams jam the rings, so the cc trigger (which barriers all 8
        # cores) fires ~16us in, aligning cores for the later a2a's.
        nc.vector.memset(wsmall[:], 0.0)
        nc.sync.dma_start(dum_in[:], wsmall[:])
        nc.gpsimd.collective_compute(
            "AllToAll", mybir.AluOpType.bypass,
            replica_groups=[list(range(N_CORES))],
            ins=[dum_in[:]], outs=[dum_out[:]])

        # wqkv chunks spread over the 3 DMA-capable rings
        nc.sync.dma_start(wqkv_sb[:, 0:4, :], wqkv_d.ap()[0])
        nc.scalar.dma_start(wqkv_sb[:, 4:8, :], wqkv_d.ap()[1])
        nc.gpsimd.dma_start(wqkv_sb[:, 8:12, :], wqkv_d.ap()[2])
        make_identity(nc, identity[:])

        dmaq = [nc.sync, nc.scalar, nc.gpsimd]

        def startup_dmas(kc):
            # interleaved behind the first x tiles so neither delays the
            # other. cos/sin go as ONE contiguous DMA each: slicing the mid
            # axis fragments into 640B descriptors that jam the ring.
            if kc == 2:
                nc.sync.dma_start(wqkv_sb[:, 12:16, :], wqkv_d.ap()[3])
            elif kc == 4:
                nc.gpsimd.dma_start(cos5_sb[:], cos5_d.ap())
                nc.gpsimd.dma_start(sin5_sb[:], sin5_d.ap())
            elif kc == 8:
                nc.scalar.dma_start(maskT_sb[:], mask_d.ap())

        # ---------------- attention machinery (used in both phases) --------
        # gpsimd cannot touch PSUM, so the exp2 bit-trick (reads scores psum)
        # only runs on the DVE.
        eng_load = {"act": 11000.0, "dve": 21000.0}

        def exp_cost(w):
            return {"act": 220 + 0.93 * w,
                    "dve": 2 * (170 + 1.0 * w)}

        def emit_probs(key, psc, po, col0, b, nb, diag, may_offload):
            """Emit the probs computation now; return a closure that emits
            the PV matmul(s) -- deferring it one block-step for sub B hides
            the DVE bit-trick's 2-op latency from the in-order PE."""
            w = 512 - col0
            if diag or not may_offload:
                eng = "act"
            else:
                cost = exp_cost(w)
                eng = min(cost, key=lambda e: eng_load[e] + cost[e])
            eng_load[eng] += exp_cost(w)[eng]
            if eng == "act":
                probs = probs_pool.tile([128, 512], BF16,
                                        name=f"pr{key}", tag="probs")
                nc.scalar.activation(probs[:, col0:512], psc[:, col0:512],
                                     AF.Exp, scale=0.125)
                if diag:
                    nc.vector.tensor_mul(probs[:, col0:col0 + 128],
                                         probs[:, col0:col0 + 128],
                                         maskT_sb[:])

                def pv():
                    nc.tensor.matmul(po[:, col0:512], v128[:, b, :],
                                     probs[:, col0:512],
                                     start=(b == 0), stop=(b == nb - 1))
            else:
                i1 = probs_pool.tile([128, 512], I16, name=f"i1{key}",
                                     tag="i1")
                i2 = probs_pool.tile([128, 512], I16, name=f"i2{key}",
                                     tag="i2")
                nc.vector.tensor_scalar(
                    i1[:, col0:512], psc[:, col0:512], SCH_A, SCH_B1,
                    mybir.AluOpType.mult, mybir.AluOpType.add)
                nc.vector.tensor_scalar(
                    i2[:, col0:512], psc[:, col0:512], SCH_A, SCH_B2,
                    mybir.AluOpType.mult, mybir.AluOpType.add)

                def pv():
                    nc.tensor.matmul(po[:, col0:512], v128[:, b, :],
                                     i1[:, col0:512].bitcast(BF16),
                                     start=(b == 0), stop=False)
                    nc.tensor.matmul(po[:, col0:512], v128b[:, b, :],
                                     i2[:, col0:512].bitcast(BF16),
                                     start=False, stop=(b == nb - 1))
            return pv

        def stage_a2a(i, t):
            # stage tile t's two 256-seq rows into DRAM as soon as the tile
            # is normalized, so only rows 6,7 + the trigger remain at the end
            for r in (2 * t, 2 * t + 1):
                nc.sync.dma_start(a2a_in[i][r],
                                  attnTs[i][:, 256 * r:256 * (r + 1)])

        def send_a2a(i):
            nc.gpsimd.collective_compute(
                "AllToAll", mybir.AluOpType.bypass,
                replica_groups=[list(range(N_CORES))],
                ins=[a2a_in[i][:]], outs=[a2a_out[i][:]])

        def finish(h, t, po):
            # den bounces through SBUF: reciprocal_approx_fast is a raw
            # bit-trick (BITWISE_NOT seed) and must read SBUF bits
            den = nrm_pool.tile([1, 512], F32, name=f"dn{h}{t}", tag="den")
            nc.scalar.copy(den[:], po[HD:HD + 1, :])
            recip = nrm_pool.tile([1, 512], F32, name=f"rc{h}{t}",
                                  tag="recip")
            nc.vector.reciprocal_approx_fast(recip[:], den[:])
            rfac = nrm_pool.tile([HD, 512], F32, name=f"rf{h}{t}", tag="rfac")
            nc.gpsimd.partition_broadcast(rfac[:], recip[:])
            nc.vector.tensor_mul(
                attnTs[h // 2][64 * (h % 2):64 * (h % 2) + HD,
                               512 * t:512 * (t + 1)],
                po[0:HD, :], rfac[:])

        def attention_steps(p, t, psc_pool, po_pool, runahead):
            """Generator: yields after each block-step so the caller can
            interleave projection work between steps (the PE queue is FIFO;
            a whole tile emitted at once stalls it on every exp)."""
            qP = qPs[p]
            nb = 4 * t + 4
            poA = po_pool.tile([128, 512], F32, name=f"poA{p}{t}", tag="poA")
            poB = po_pool.tile([128, 512], F32, name=f"poB{p}{t}", tag="poB")
            pscs = {}

            def scores(b):
                j = max(0, b - 4 * t)
                col0 = 128 * j
                pscA = psc_pool.tile([128, 512], F32, name=f"pA{p}{t}{b}",
                                     tag="pscA")
                pscB = psc_pool.tile([128, 512], F32, name=f"pB{p}{t}{b}",
                                     tag="pscB")
                qc = (512 * t + col0, 512 * (t + 1))
                kc_ = (128 * b, 128 * (b + 1))
                # concurrent in the PE: rows 0:63 and 64:127
                nc.tensor.matmul(pscA[:, col0:512], kT[0:64, kc_[0]:kc_[1]],
                                 qP[0:64, qc[0]:qc[1]], start=True, stop=True)
                nc.tensor.matmul(pscB[:, col0:512], kT[64:128, kc_[0]:kc_[1]],
                                 qP[64:128, qc[0]:qc[1]],
                                 start=True, stop=True)
                pscs[b] = (pscA, pscB, col0)

            for b in range(min(runahead + 1, nb)):
                scores(b)
            pend_b = None
            for b in range(nb):
                if b + runahead + 1 < nb:
                    scores(b + runahead + 1)
                pscA, pscB, col0 = pscs.pop(b)
                diag = b >= 4 * t
                if pend_b is not None:
                    pend_b()
                pvA = emit_probs(f"{p}{t}{b}0", pscA, poA, col0,
                                 b, nb, diag, may_offload=False)
                pvA()
                pend_b = emit_probs(f"{p}{t}{b}1", pscB, poB, col0,
                                    b, nb, diag, may_offload=True)
                yield
            pend_b()
            finish(2 * p, t, poA)
            finish(2 * p + 1, t, poB)
            stage_a2a(p, t)

        def attention_tile(p, t, psc_pool, po_pool, runahead):
            for _ in attention_steps(p, t, psc_pool, po_pool, runahead):
                pass

        # ------- Stage P (+ BOTH pairs' attention t=0..2 interleaved) ------
        with ExitStack() as ctx:
            xtb_pool = ctx.enter_context(tc.tile_pool(name="xtb", bufs=20))
            S_pool = ctx.enter_context(
                tc.tile_pool(name="Sp", bufs=1, space="PSUM"))
            pvt_pool = ctx.enter_context(
                tc.tile_pool(name="pvt", bufs=1, space="PSUM"))
            tmp_pool = ctx.enter_context(tc.tile_pool(name="ropetmp", bufs=2))
            pscI_pool = ctx.enter_context(
                tc.tile_pool(name="pscI", bufs=2, space="PSUM"))
            poI_pool = ctx.enter_context(
                tc.tile_pool(name="poI", bufs=1, space="PSUM"))

            nc.vector.memset(v128[:, :, HD:HD + 1], 1.0)
            nc.vector.memset(v128[:, :, HD + 1:], 0.0)
            nc.vector.memset(v128b[:, :, HD:HD + 1], 0.70710678)
            nc.vector.memset(v128b[:, :, HD + 1:], 0.0)

            def evens(ap):
                return ap.rearrange("p (n two) -> p two n", two=2)[:, 0, :]

            def odds(ap):
                return ap.rearrange("p (n two) -> p two n", two=2)[:, 1, :]

            def rope_block(blk, Sp):
                cosb = cos5_sb[:, blk, :]
                sinb = sin5_sb[:, blk, :]
                Se = evens(Sp[:, 0:RCOLS])
                So = odds(Sp[:, 0:RCOLS])
                t1 = tmp_pool.tile([128, RCOLS // 2], F32, name=f"t1_{blk}",
                                   tag="t1")
                t2 = tmp_pool.tile([128, RCOLS // 2], F32, name=f"t2_{blk}",
                                   tag="t2")
                t3 = tmp_pool.tile([128, RCOLS // 2], F32, name=f"t3_{blk}",
                                   tag="t3")
                t4 = tmp_pool.tile([128, RCOLS // 2], F32, name=f"t4_{blk}",
                                   tag="t4")
                stg = tmp_pool.tile([128, RCOLS], F32, name=f"stg_{blk}",
                                    tag="stg")
                nc.vector.tensor_mul(t1[:], Se, cosb)
                nc.vector.tensor_mul(t2[:], So, sinb)
                nc.vector.tensor_mul(t3[:], Se, sinb)
                nc.vector.tensor_mul(t4[:], So, cosb)
                nc.gpsimd.tensor_sub(evens(stg[:]), t1[:], t2[:])
                nc.gpsimd.tensor_add(odds(stg[:]), t3[:], t4[:])
                # v drops straight into PV-lhs orientation
                nc.scalar.copy(v128[:, blk, 0:HD], Sp[:, RCOLS:PCOLS])
                nc.scalar.mul(v128b[:, blk, 0:HD], Sp[:, RCOLS:PCOLS],
                              0.70710678)
                # transpose roped q (2x128 dims) and k (64) into column-major
                for g in range(2):
                    pvq = pvt_pool.tile([128, 128], F32, name=f"pvq{blk}{g}",
                                        tag="pvq")
                    nc.tensor.transpose(pvq[:], stg[:, 128 * g:128 * (g + 1)],
                                        identity[:])
                    nc.scalar.copy(qPs[g][:, 128 * blk:128 * (blk + 1)],
                                   pvq[:])
                # k transpose shares the pvq tag/bank (rows 0:64) so pvt
                # stays a single PSUM bank
                pvk = pvt_pool.tile([128, 128], F32, name=f"pvk{blk}",
                                    tag="pvq")
                nc.tensor.transpose(pvk[0:HD, :], stg[:, QCOLS:RCOLS],
                                    identity[:])
                nc.scalar.copy(kT[0:HD, 128 * blk:128 * (blk + 1)],
                               pvk[0:HD, :])
                nc.scalar.copy(kT[HD:128, 128 * blk:128 * (blk + 1)],
                               pvk[0:HD, :])

            # BOTH pairs' attention tiles t=0..2 ride the DMA slack of stage
            # P (kT/v are shared by the two pairs; tile t only needs seq
            # blocks 0..4t+3). Sequential generators keep PSUM at 7 banks:
            # S 2 + pvt 1 + pscA/B 2 + poA/B 2.
            gens = []

            def pump(n):
                done = 0
                while gens and done < n:
                    try:
                        next(gens[0])
                        done += 1
                    except StopIteration:
                        gens.pop(0)
                return done

            for sq2 in range(2):
                xtbs = []
                for kc in range(16):
                    xtb = xtb_pool.tile([128, 1024], BF16,
                                        name=f"xtb{sq2}_{kc}", tag="xtb")
                    dmaq[kc % 3].dma_start(xtb[:], xT_d.ap()[sq2, kc])
                    xtbs.append(xtb)
                    if sq2 == 0:
                        startup_dmas(kc)
                for j in range(8):
                    blk = 8 * sq2 + j
                    Sp = S_pool.tile([128, PCOLS], F32, name=f"S{blk}",
                                     tag="S")
                    for kc in range(16):
                        nc.tensor.matmul(
                            Sp[:], xtbs[kc][:, 128 * j:128 * (j + 1)],
                            wqkv_sb[:, kc, :],
                            start=(kc == 0), stop=(kc == 15))
                        if kc % 3 == 2:
                            pump(1)
                    rope_block(blk, Sp)
                    if blk % 4 == 3 and blk < 15:
                        gens.append(attention_steps(0, blk // 4, pscI_pool,
                                                    poI_pool, runahead=1))
                        gens.append(attention_steps(1, blk // 4, pscI_pool,
                                                    poI_pool, runahead=1))
            while pump(1):
                pass

            # wo prefetch for stage W; off the sync ring so the a2a_in
            # staging DMAs aren't queued behind it
            nc.scalar.dma_start(wo_sb[:, 0:8, :], wo_d.ap()[:, 0:8, :])
            nc.gpsimd.dma_start(wo_sb[:, 8:16, :], wo_d.ap()[:, 8:16, :])

        # --------- Tail: the two t=3 tiles (need proj block 15) + A2As ----
        with ExitStack() as ctx:
            psc_pool = ctx.enter_context(
                tc.tile_pool(name="psc", bufs=2, space="PSUM"))
            po_pool = ctx.enter_context(
                tc.tile_pool(name="po", bufs=2, space="PSUM"))
            attention_tile(0, 3, psc_pool, po_pool, runahead=1)
            send_a2a(0)          # cc runs while pair-1 t3 computes
            attention_tile(1, 3, psc_pool, po_pool, runahead=1)
            send_a2a(1)

        # Stage W: out rows = attn_fullT.T @ wo, accumulated in two passes
        # (even h-chunks from a2a chunk 0, odd from chunk 1). The i=1 pass is
        # m-outer so m=0's psum copy + out DMA overlap m=1's matmuls.
        with ExitStack() as ctx:
            af_pool = ctx.enter_context(tc.tile_pool(name="af", bufs=1))
            pw_pool = ctx.enter_context(
                tc.tile_pool(name="pw", bufs=1, space="PSUM"))
            osb_pool = ctx.enter_context(tc.tile_pool(name="osb", bufs=2))
            afs = []
            for i, eng in ((0, nc.sync), (1, nc.scalar)):
                af = af_pool.tile([128, N_CORES, ROWS_PER_CORE], BF16,
                                  name=f"attn_full{i}", uniquify=False)
                eng.dma_start(af[:],
                              a2a_out[i][:].rearrange("r p s -> p r s"))
                afs.append(af)
            pw = [[pw_pool.tile([128, 512], F32, name=f"pw{m}{n}",
                                tag=f"pw{m}{n}") for n in range(4)]
                  for m in range(2)]
            for r in range(N_CORES):          # i = 0: even h-chunks
                for m in range(2):
                    lhs = afs[0][:, r, 128 * m:128 * (m + 1)]
                    for n in range(4):
                        nc.tensor.matmul(
                            pw[m][n][:], lhs,
                            wo_sb[:, 2 * r, 512 * n:512 * (n + 1)],
                            start=(r == 0), stop=False)
            for m in range(2):                # i = 1: odd h-chunks, m-outer
                for r in range(N_CORES):
                    lhs = afs[1][:, r, 128 * m:128 * (m + 1)]
                    for n in range(4):
                        nc.tensor.matmul(
                            pw[m][n][:], lhs,
                            wo_sb[:, 2 * r + 1, 512 * n:512 * (n + 1)],
                            start=False, stop=(r == N_CORES - 1))
                osb = osb_pool.tile([128, D], F32, name=f"osb{m}", tag="osb")
                for n in range(4):
                    nc.scalar.copy(osb[:, 512 * n:512 * (n + 1)], pw[m][n][:])
                    eng = nc.sync if n % 2 == 0 else nc.scalar
                    eng.dma_start(
                        out_d.ap()[128 * m:128 * (m + 1),
                                   512 * n:512 * (n + 1)],
                        osb[:, 512 * n:512 * (n + 1)])

    nc.compile()
    return nc


_NC_CACHE = None
LAST_RESULT = None


def _get_nc():
    global _NC_CACHE
    if _NC_CACHE is None:
        _NC_CACHE = _build()
    return _NC_CACHE


def kernel(x, wq, wk, wv, wo, freqs_cos, freqs_sin, mask, start_pos=0):
    assert int(start_pos) == 0, "kernel specialized for start_pos == 0"
    import ml_dtypes
    x = np.asarray(x, np.float32)
    b, s, d = x.shape
    assert (b, s, d) == (1, S, D)
    xT = np.ascontiguousarray(x[0].T).astype(ml_dtypes.bfloat16)
    # pre-tile: xT[sq2, kc] = contiguous (128, 1024) block -> 2KB DMA lines
    xTt = np.ascontiguousarray(
        xT.reshape(16, 128, 2, 1024).transpose(2, 0, 1, 3))
    wq = np.asarray(wq, np.float32)
    wk = np.asarray(wk, np.float32)
    wv = np.asarray(wv, np.float32)
    wot = np.ascontiguousarray(
        np.asarray(wo, np.float32).reshape(16, 128, D).transpose(1, 0, 2)
    ).astype(ml_dtypes.bfloat16)
    cos = np.asarray(freqs_cos, np.float32)   # (S, 32)
    sin = np.asarray(freqs_sin, np.float32)
    cos5 = np.ascontiguousarray(
        np.tile(cos, (1, 5)).reshape(16, 128, RCOLS // 2).transpose(1, 0, 2)
    ).astype(ml_dtypes.bfloat16)
    sin5 = np.ascontiguousarray(
        np.tile(sin, (1, 5)).reshape(16, 128, RCOLS // 2).transpose(1, 0, 2)
    ).astype(ml_dtypes.bfloat16)
    maskT01 = np.ascontiguousarray(
        (np.asarray(mask, np.float32)[:128, :128].T == 0.0)
    ).astype(ml_dtypes.bfloat16)

    in_maps = []
    for c in range(N_CORES):
        wqkv_c = np.concatenate(
            [wq[:, QCOLS * c:QCOLS * (c + 1)],
             wk[:, HD * c:HD * (c + 1)],
             wv[:, HD * c:HD * (c + 1)]], axis=1)   # (2048, 384)
        # [g][p][kc_in_g][col]
        wqkv_t = np.ascontiguousarray(
            wqkv_c.reshape(4, 4, 128, PCOLS).transpose(0, 2, 1, 3)
        ).astype(ml_dtypes.bfloat16)
        in_maps.append({
            "xT": xTt,
            "wqkv": wqkv_t,
            "wo": wot,
            "cos5": cos5,
            "sin5": sin5,
            "maskT01": maskT01,
        })

    nc = _get_nc()
    res = bass_utils.run_bass_kernel_spmd(
        nc, in_maps, core_ids=list(range(N_CORES)),
        trace=bool(os.environ.get("BASS_TRACE")))
    global LAST_RESULT
    LAST_RESULT = res
    rows = [res.results[c]["out"] for c in range(N_CORES)]
    return np.concatenate(rows, axis=0).reshape(1, S, D).astype(np.float32)



# revision 12
# speedup vs baseline: 1.1296x; 1.1296x over previous
"""GQA attention (S=2048, D=2048, 32 q-heads / 8 kv-heads, rope, causal) on 8
Trainium2 NeuronCores, tensor-parallel over heads (1 kv head + 4 q heads per
core), chunked AllToAll re-shard overlapped with compute, row-sharded output.

v4 on top of v3 (trace-driven):
 - cos/sin/mask load as single contiguous DMAs (the chunked mid-axis slices
   fragmented into 640B descriptors that jammed the rings and stalled the
   PE ~40us waiting for x chunks queued behind them).
 - warm-up collective staged first on the sync ring -> cores barrier-align
   ~16us in instead of ~66us.
 - BOTH head-pairs' attention tiles t=0..2 interleave into stage P (kT/v
   are shared by the pairs); only the two t=3 tiles remain after block 15,
   so a2a(0) fires ~8us after stage P and a2a(1) ~8us later, with pair-1
   t3 and stage-W i=0 overlapping the collectives.
 - a2a staging DMAs issue incrementally as each tile is normalized.

v3 design vs the original baseline (408us):
 - stage P computes projections UNTRANSPOSED (seq on partitions, qkv dims on
   the free axis, one fused (128,384) q|k|v psum tile per 128-seq block).
   Rope pairs are then adjacent on the free axis, so the rotation runs as
   full-width (128,160) stride-2 vector ops instead of 30 (32,512)-shaped
   ops per seq tile. v lands directly in PV-lhs orientation; q/k are
   transposed by the PE via identity matmuls.
 - stage P is DMA-floor-paced (x streams ~9MB over 3 rings), so the first
   head-pair's ATTENTION IS INTERLEAVED INTO stage P: attention q-tile t
   only needs seq blocks 0..4t+3, so it runs right after P block 4t+3,
   filling the PE/ACT/DVE idle time under the DMA wait and firing the first
   AllToAll ~80us earlier. This also keeps the PE continuously busy -- its
   clock is activity-gated (HAM throttles to 1.2GHz when idling).
 - scores matmuls for the two heads of a pair run CONCURRENTLY in the PE
   array via row tiling (head A in rows 0:63 / tile_position (0,0), head B
   in rows 64:127 / (64,0)); kT is duplicated into partitions 64:128.
 - exp: diagonal tiles use exact ACT-table exp; full-width tiles are
   balanced between ACT and a paired-Schraudolph exp2 bit trick on the DVE
   (two f32->int16 tensor_scalar ops bitcast to bf16; the two PV matmuls
   against v and v/sqrt2 average the pair, cancelling most of the sawtooth
   error inside the softmax). Sub-B PV matmuls lag one block-step so the
   DVE latency never stalls the in-order PE.
 - PV matmuls use the full 128-col v tile (cols 65:128 zero) so fast weight
   load kicks in; the ones-column at col 64 yields softmax denominators.
 - stage W: i=1 pass is m-outer so the m=0 psum copy + output DMA overlap
   the m=1 matmuls.

Self-contained: takes full inputs, shards on host, runs one SPMD Bass/Tile
kernel via run_bass_kernel_spmd, reassembles the full output.
"""
import os
import sys
from contextlib import ExitStack

import numpy as np

try:
    import concourse.bass as bass  # noqa: F401
except ImportError:  # platform tree not on sys.path in a fresh dir
    sys.path.insert(0, "/opt/trn_rl_repo")
    import concourse.bass as bass  # noqa: F401

import concourse.mybir as mybir
from concourse import bacc, bass_utils, tile
from concourse.masks import make_identity

F32 = mybir.dt.float32
BF16 = mybir.dt.bfloat16
I16 = mybir.dt.int16
AF = mybir.ActivationFunctionType

S = 2048          # sequence length
D = 2048          # model dim
HD = 64           # head dim
N_CORES = 8
QH_PER_CORE = 4   # q heads per core (32/8)
QCOLS = QH_PER_CORE * HD       # 256 q-projection cols per core
PCOLS = QCOLS + 2 * HD         # 384 fused q|k|v cols per core
RCOLS = QCOLS + HD             # 320 roped cols (q|k)
ROWS_PER_CORE = S // N_CORES   # 256 output rows per core

SCH_A = 0.125 * 128 * 1.4426950408889634
# bias: 127*128 maps the exponent; -128 halves (the pair S1+S2/sqrt2 would
# otherwise sum to ~2*exp, mismatching exact-exp blocks in the same softmax
# chain); -6.87 removes the pair's +3.85% mean sawtooth bias so approx and
# exact blocks carry equal weight in the denominator.
SCH_B1 = 127.0 * 128.0 - 128.0 - 6.87
SCH_B2 = SCH_B1 + 64.0


def _build():
    nc = bacc.Bacc("TRN2", target_bir_lowering=False, debug=False,
                   num_devices=N_CORES)
    xT_d = nc.dram_tensor("xT", [2, 16, 128, 1024], BF16, kind="ExternalInput")
    wqkv_d = nc.dram_tensor("wqkv", [4, 128, 4, PCOLS], BF16,
                            kind="ExternalInput")
    wo_d = nc.dram_tensor("wo", [128, 16, D], BF16, kind="ExternalInput")
    cos5_d = nc.dram_tensor("cos5", [128, 16, RCOLS // 2], BF16,
                            kind="ExternalInput")
    sin5_d = nc.dram_tensor("sin5", [128, 16, RCOLS // 2], BF16,
                            kind="ExternalInput")
    mask_d = nc.dram_tensor("maskT01", [128, 128], BF16, kind="ExternalInput")
    out_d = nc.dram_tensor("out", [ROWS_PER_CORE, D], F32, kind="ExternalOutput")

    with tile.TileContext(nc) as tc, ExitStack() as top:
        persist = top.enter_context(tc.tile_pool(name="persist", bufs=1))
        # head-pair q tiles: rows = [head(2p) dims | head(2p+1) dims]
        qPs = [persist.tile([128, S], BF16, name=f"qP{i}", uniquify=False)
               for i in range(2)]
        # k dims on rows 0:64, duplicated to 64:128 for the paired scores mm
        kT = persist.tile([128, S], BF16, name="kT")
        v128 = persist.tile([128, 16, 128], BF16, name="v128")
        # v/sqrt2: second PV operand of the paired-Schraudolph approx path
        v128b = persist.tile([128, 16, 128], BF16, name="v128b")
        attnT0 = persist.tile([128, S], BF16, name="attnT0")
        attnT1 = persist.tile([128, S], BF16, name="attnT1")
        attnTs = [attnT0, attnT1]
        maskT_sb = persist.tile([128, 128], BF16, name="maskT_sb")
        cos5_sb = persist.tile([128, 16, RCOLS // 2], F32, name="cos5_sb")
        sin5_sb = persist.tile([128, 16, RCOLS // 2], F32, name="sin5_sb")
        wqkv_sb = persist.tile([128, 16, PCOLS], BF16, name="wqkv_sb")
        wo_sb = persist.tile([128, 16, D], BF16, name="wo_sb")
        identity = persist.tile([128, 128], F32, name="identity")
        wsmall = persist.tile([8, 64], BF16, name="wsmall")
        probs_pool = top.enter_context(tc.tile_pool(name="probs", bufs=3))
        nrm_pool = top.enter_context(tc.tile_pool(name="nrm", bufs=4))

        dram = top.enter_context(tc.tile_pool(name="dram", bufs=1, space="DRAM"))
        a2a_in = [dram.tile([N_CORES, 128, ROWS_PER_CORE], BF16,
                            name=f"a2a_in{i}", uniquify=False)
                  for i in range(2)]
        a2a_out = [dram.tile([N_CORES, 128, ROWS_PER_CORE], BF16,
                             name=f"a2a_out{i}", uniquify=False)
                   for i in range(2)]
        dum_in = dram.tile([N_CORES, 64], BF16, name="dum_in", uniquify=False)
        dum_out = dram.tile([N_CORES, 64], BF16, name="dum_out",
                            uniquify=False)

        # warm-up collective FIRST: its 1KB staging lands before the big
        # streams jam the rings, so the cc trigger (which barriers all 8
        # cores) fires ~16us in, aligning cores for the later a2a's.
        nc.vector.memset(wsmall[:], 0.0)
        nc.sync.dma_start(dum_in[:], wsmall[:])
        nc.gpsimd.collective_compute(
            "AllToAll", mybir.AluOpType.bypass,
            replica_groups=[list(range(N_CORES))],
            ins=[dum_in[:]], outs=[dum_out[:]])

        # wqkv chunks spread over the 3 DMA-capable rings
        nc.sync.dma_start(wqkv_sb[:, 0:4, :], wqkv_d.ap()[0])
        nc.scalar.dma_start(wqkv_sb[:, 4:8, :], wqkv_d.ap()[1])
        nc.gpsimd.dma_start(wqkv_sb[:, 8:12, :], wqkv_d.ap()[2])
        make_identity(nc, identity[:])

        dmaq = [nc.sync, nc.scalar, nc.gpsimd]

        def startup_dmas(kc):
            # interleaved behind the first x tiles so neither delays the
            # other. cos/sin go as ONE contiguous DMA each: slicing the mid
            # axis fragments into 640B descriptors that jam the ring.
            if kc == 2:
                nc.sync.dma_start(wqkv_sb[:, 12:16, :], wqkv_d.ap()[3])
            elif kc == 4:
                nc.gpsimd.dma_start(cos5_sb[:], cos5_d.ap())
                nc.gpsimd.dma_start(sin5_sb[:], sin5_d.ap())
            elif kc == 8:
                nc.scalar.dma_start(maskT_sb[:], mask_d.ap())

        # ---------------- attention machinery (used in both phases) --------
        # gpsimd cannot touch PSUM, so the exp2 bit-trick (reads scores psum)
        # only runs on the DVE.
        eng_load = {"act": 11000.0, "dve": 21000.0}

        def exp_cost(w):
            return {"act": 220 + 0.93 * w,
                    "dve": 2 * (170 + 1.0 * w)}

        def emit_probs(key, psc, po, col0, b, nb, diag, may_offload):
            """Emit the probs computation now; return a closure that emits
            the PV matmul(s) -- deferring it one block-step for sub B hides
            the DVE bit-trick's 2-op latency from the in-order PE."""
            w = 512 - col0
            if diag or not may_offload:
                eng = "act"
            else:
                cost = exp_cost(w)
                eng = min(cost, key=lambda e: eng_load[e] + cost[e])
            eng_load[eng] += exp_cost(w)[eng]
            if eng == "act":
                probs = probs_pool.tile([128, 512], BF16,
                                        name=f"pr{key}", tag="probs")
                nc.scalar.activation(probs[:, col0:512], psc[:, col0:512],
                                     AF.Exp, scale=0.125)
                if diag:
                    nc.vector.tensor_mul(probs[:, col0:col0 + 128],
                                         probs[:, col0:col0 + 128],
                                         maskT_sb[:])

                def pv():
                    nc.tensor.matmul(po[:, col0:512], v128[:, b, :],
                                     probs[:, col0:512],
                                     start=(b == 0), stop=(b == nb - 1))
            else:
                i1 = probs_pool.tile([128, 512], I16, name=f"i1{key}",
                                     tag="i1")
                i2 = probs_pool.tile([128, 512], I16, name=f"i2{key}",
                                     tag="i2")
                nc.vector.tensor_scalar(
                    i1[:, col0:512], psc[:, col0:512], SCH_A, SCH_B1,
                    mybir.AluOpType.mult, mybir.AluOpType.add)
                nc.vector.tensor_scalar(
                    i2[:, col0:512], psc[:, col0:512], SCH_A, SCH_B2,
                    mybir.AluOpType.mult, mybir.AluOpType.add)

                def pv():
                    nc.tensor.matmul(po[:, col0:512], v128[:, b, :],
                                     i1[:, col0:512].bitcast(BF16),
                                     start=(b == 0), stop=False)
                    nc.tensor.matmul(po[:, col0:512], v128b[:, b, :],
                                     i2[:, col0:512].bitcast(BF16),
                                     start=False, stop=(b == nb - 1))
            return pv

        def stage_a2a(i, t):
            # stage tile t's two 256-seq rows into DRAM as soon as the tile
            # is normalized, so only rows 6,7 + the trigger remain at the end
            for r in (2 * t, 2 * t + 1):
                nc.sync.dma_start(a2a_in[i][r],
                                  attnTs[i][:, 256 * r:256 * (r + 1)])

        def send_a2a(i):
            nc.gpsimd.collective_compute(
                "AllToAll", mybir.AluOpType.bypass,
                replica_groups=[list(range(N_CORES))],
                ins=[a2a_in[i][:]], outs=[a2a_out[i][:]])

        def finish(h, t, po):
            # den bounces through SBUF: reciprocal_approx_fast is a raw
            # bit-trick (BITWISE_NOT seed) and must read SBUF bits
            den = nrm_pool.tile([1, 512], F32, name=f"dn{h}{t}", tag="den")
            nc.scalar.copy(den[:], po[HD:HD + 1, :])
            recip = nrm_pool.tile([1, 512], F32, name=f"rc{h}{t}",
                                  tag="recip")
            nc.vector.reciprocal_approx_fast(recip[:], den[:])
            rfac = nrm_pool.tile([HD, 512], F32, name=f"rf{h}{t}", tag="rfac")
            nc.gpsimd.partition_broadcast(rfac[:], recip[:])
            nc.vector.tensor_mul(
                attnTs[h // 2][64 * (h % 2):64 * (h % 2) + HD,
                               512 * t:512 * (t + 1)],
                po[0:HD, :], rfac[:])

        def attention_steps(p, t, pscA_pool, pscB_pool, po_pool,
                            runahead):
            """Generator: yields after each block-step so the caller can
            interleave projection work between steps (the PE queue is FIFO;
            a whole tile emitted at once stalls it on every exp)."""
            qP = qPs[p]
            nb = 4 * t + 4
            poA = po_pool.tile([128, 512], F32, name=f"poA{p}{t}", tag="poA")
            poB = po_pool.tile([128, 512], F32, name=f"poB{p}{t}", tag="poB")
            pscs = {}

            def scores(b):
                j = max(0, b - 4 * t)
                col0 = 128 * j
                pscA = pscA_pool.tile([128, 512], F32,
                                      name=f"pA{p}{t}{b}", tag="pscA")
                pscB = pscB_pool.tile([128, 512], F32,
                                      name=f"pB{p}{t}{b}", tag="pscB")
                qc = (512 * t + col0, 512 * (t + 1))
                kc_ = (128 * b, 128 * (b + 1))
                # concurrent in the PE: rows 0:63 and 64:127
                nc.tensor.matmul(pscA[:, col0:512], kT[0:64, kc_[0]:kc_[1]],
                                 qP[0:64, qc[0]:qc[1]], start=True, stop=True)
                nc.tensor.matmul(pscB[:, col0:512], kT[64:128, kc_[0]:kc_[1]],
                                 qP[64:128, qc[0]:qc[1]],
                                 start=True, stop=True)
                pscs[b] = (pscA, pscB, col0)

            for b in range(min(runahead + 1, nb)):
                scores(b)
            pend_b = None
            for b in range(nb):
                pscA, pscB, col0 = pscs.pop(b)
                diag = b >= 4 * t
                if pend_b is not None:
                    pend_b()
                pvA = emit_probs(f"{p}{t}{b}0", pscA, poA, col0,
                                 b, nb, diag, may_offload=False)
                pvA()
                pend_b = emit_probs(f"{p}{t}{b}1", pscB, poB, col0,
                                    b, nb, diag, may_offload=True)
                # scores for the lookahead step go LAST: the PE queue is
                # in-order, so a WAR-stalled scores mm must not block the
                # ready PV/probs work above
                if b + runahead + 1 < nb:
                    scores(b + runahead + 1)
                yield
            pend_b()
            finish(2 * p, t, poA)
            finish(2 * p + 1, t, poB)
            stage_a2a(p, t)

        def attention_tile(p, t, pscA_pool, pscB_pool, po_pool,
                           runahead):
            for _ in attention_steps(p, t, pscA_pool, pscB_pool, po_pool,
                                     runahead):
                pass

        # ------- Stage P (+ BOTH pairs' attention t=0..2 interleaved) ------
        with ExitStack() as ctx:
            xtb_pool = ctx.enter_context(tc.tile_pool(name="xtb", bufs=20))
            S_pool = ctx.enter_context(
                tc.tile_pool(name="Sp", bufs=2, space="PSUM"))
            pvt_pool = ctx.enter_context(
                tc.tile_pool(name="pvt", bufs=1, space="PSUM"))
            tmp_pool = ctx.enter_context(tc.tile_pool(name="ropetmp", bufs=2))
            pscIA_pool = ctx.enter_context(
                tc.tile_pool(name="pscIA", bufs=2, space="PSUM"))
            pscIB_pool = ctx.enter_context(
                tc.tile_pool(name="pscIB", bufs=1, space="PSUM"))
            poI_pool = ctx.enter_context(
                tc.tile_pool(name="poI", bufs=1, space="PSUM"))

            nc.vector.memset(v128[:, :, HD:HD + 1], 1.0)
            nc.vector.memset(v128[:, :, HD + 1:], 0.0)
            nc.vector.memset(v128b[:, :, HD:HD + 1], 0.70710678)
            nc.vector.memset(v128b[:, :, HD + 1:], 0.0)

            def evens(ap):
                return ap.rearrange("p (n two) -> p two n", two=2)[:, 0, :]

            def odds(ap):
                return ap.rearrange("p (n two) -> p two n", two=2)[:, 1, :]

            def rope_block(blk, Sp):
                cosb = cos5_sb[:, blk, :]
                sinb = sin5_sb[:, blk, :]
                Se = evens(Sp[:, 0:RCOLS])
                So = odds(Sp[:, 0:RCOLS])
                t1 = tmp_pool.tile([128, RCOLS // 2], F32, name=f"t1_{blk}",
                                   tag="t1")
                t2 = tmp_pool.tile([128, RCOLS // 2], F32, name=f"t2_{blk}",
                                   tag="t2")
                t3 = tmp_pool.tile([128, RCOLS // 2], F32, name=f"t3_{blk}",
                                   tag="t3")
                t4 = tmp_pool.tile([128, RCOLS // 2], F32, name=f"t4_{blk}",
                                   tag="t4")
                stg = tmp_pool.tile([128, RCOLS], F32, name=f"stg_{blk}",
                                    tag="stg")
                nc.vector.tensor_mul(t1[:], Se, cosb)
                nc.vector.tensor_mul(t2[:], So, sinb)
                nc.vector.tensor_mul(t3[:], Se, sinb)
                nc.vector.tensor_mul(t4[:], So, cosb)
                nc.gpsimd.tensor_sub(evens(stg[:]), t1[:], t2[:])
                nc.gpsimd.tensor_add(odds(stg[:]), t3[:], t4[:])
                # v drops straight into PV-lhs orientation
                nc.scalar.copy(v128[:, blk, 0:HD], Sp[:, RCOLS:PCOLS])
                nc.scalar.mul(v128b[:, blk, 0:HD], Sp[:, RCOLS:PCOLS],
                              0.70710678)
                # transpose roped q (2x128 dims) and k (64) into column-major
                for g in range(2):
                    pvq = pvt_pool.tile([128, 128], F32, name=f"pvq{blk}{g}",
                                        tag="pvq")
                    nc.tensor.transpose(pvq[:], stg[:, 128 * g:128 * (g + 1)],
                                        identity[:])
                    nc.scalar.copy(qPs[g][:, 128 * blk:128 * (blk + 1)],
                                   pvq[:])
                # k transpose shares the pvq tag/bank (rows 0:64) so pvt
                # stays a single PSUM bank
                pvk = pvt_pool.tile([128, 128], F32, name=f"pvk{blk}",
                                    tag="pvq")
                nc.tensor.transpose(pvk[0:HD, :], stg[:, QCOLS:RCOLS],
                                    identity[:])
                nc.scalar.copy(kT[0:HD, 128 * blk:128 * (blk + 1)],
                               pvk[0:HD, :])
                nc.scalar.copy(kT[HD:128, 128 * blk:128 * (blk + 1)],
                               pvk[0:HD, :])

            # BOTH pairs' attention tiles t=0..2 ride the DMA slack of stage
            # P (kT/v are shared by the two pairs; tile t only needs seq
            # blocks 0..4t+3). Sequential generators keep PSUM at 7 banks:
            # S 2 + pvt 1 + pscA/B 2 + poA/B 2.
            gens = []

            def pump(n):
                done = 0
                while gens and done < n:
                    try:
                        next(gens[0])
                        done += 1
                    except StopIteration:
                        gens.pop(0)
                return done

            for sq2 in range(2):
                xtbs = []
                for kc in range(16):
                    xtb = xtb_pool.tile([128, 1024], BF16,
                                        name=f"xtb{sq2}_{kc}", tag="xtb")
                    dmaq[kc % 3].dma_start(xtb[:], xT_d.ap()[sq2, kc])
                    xtbs.append(xtb)
                    if sq2 == 0:
                        startup_dmas(kc)
                for j in range(8):
                    blk = 8 * sq2 + j
                    Sp = S_pool.tile([128, PCOLS], F32, name=f"S{blk}",
                                     tag="S")
                    for kc in range(16):
                        nc.tensor.matmul(
                            Sp[:], xtbs[kc][:, 128 * j:128 * (j + 1)],
                            wqkv_sb[:, kc, :],
                            start=(kc == 0), stop=(kc == 15))
                        if kc % 3 == 2:
                            pump(1)
                    rope_block(blk, Sp)
                    if blk % 4 == 3 and blk < 15:
                        gens.append(attention_steps(
                            0, blk // 4, pscIA_pool, pscIB_pool, poI_pool,
                            runahead=1))
                        gens.append(attention_steps(
                            1, blk // 4, pscIA_pool, pscIB_pool, poI_pool,
                            runahead=1))
            while pump(1):
                pass

            # wo prefetch for stage W; off the sync ring so the a2a_in
            # staging DMAs aren't queued behind it
            nc.scalar.dma_start(wo_sb[:, 0:8, :], wo_d.ap()[:, 0:8, :])
            nc.gpsimd.dma_start(wo_sb[:, 8:16, :], wo_d.ap()[:, 8:16, :])

        # --------- Tail: the two t=3 tiles (need proj block 15) + A2As ----
        with ExitStack() as ctx:
            pscA_pool = ctx.enter_context(
                tc.tile_pool(name="pscA", bufs=2, space="PSUM"))
            pscB_pool = ctx.enter_context(
                tc.tile_pool(name="pscB", bufs=2, space="PSUM"))
            po_pool = ctx.enter_context(
                tc.tile_pool(name="po", bufs=2, space="PSUM"))
            attention_tile(0, 3, pscA_pool, pscB_pool, po_pool, runahead=1)
            send_a2a(0)          # cc runs while pair-1 t3 computes
            attention_tile(1, 3, pscA_pool, pscB_pool, po_pool, runahead=1)
            send_a2a(1)

        # Stage W: out rows = attn_fullT.T @ wo, accumulated in two passes
        # (even h-chunks from a2a chunk 0, odd from chunk 1). The i=1 pass is
        # m-outer so m=0's psum copy + out DMA overlap m=1's matmuls.
        with ExitStack() as ctx:
            af_pool = ctx.enter_context(tc.tile_pool(name="af", bufs=1))
            pw_pool = ctx.enter_context(
                tc.tile_pool(name="pw", bufs=1, space="PSUM"))
            osb_pool = ctx.enter_context(tc.tile_pool(name="osb", bufs=2))
            afs = []
            for i, eng in ((0, nc.sync), (1, nc.scalar)):
                af = af_pool.tile([128, N_CORES, ROWS_PER_CORE], BF16,
                                  name=f"attn_full{i}", uniquify=False)
                eng.dma_start(af[:],
                              a2a_out[i][:].rearrange("r p s -> p r s"))
                afs.append(af)
            pw = [[pw_pool.tile([128, 512], F32, name=f"pw{m}{n}",
                                tag=f"pw{m}{n}") for n in range(4)]
                  for m in range(2)]
            for r in range(N_CORES):          # i = 0: even h-chunks
                for m in range(2):
                    lhs = afs[0][:, r, 128 * m:128 * (m + 1)]
                    for n in range(4):
                        nc.tensor.matmul(
                            pw[m][n][:], lhs,
                            wo_sb[:, 2 * r, 512 * n:512 * (n + 1)],
                            start=(r == 0), stop=False)
            for m in range(2):                # i = 1: odd h-chunks, m-outer
                for r in range(N_CORES):
                    lhs = afs[1][:, r, 128 * m:128 * (m + 1)]
                    for n in range(4):
                        nc.tensor.matmul(
                            pw[m][n][:], lhs,
                            wo_sb[:, 2 * r + 1, 512 * n:512 * (n + 1)],
                            start=False, stop=(r == N_CORES - 1))
                osb = osb_pool.tile([128, D], F32, name=f"osb{m}", tag="osb")
                for n in range(4):
                    nc.scalar.copy(osb[:, 512 * n:512 * (n + 1)], pw[m][n][:])
                    eng = nc.sync if n % 2 == 0 else nc.scalar
                    eng.dma_start(
                        out_d.ap()[128 * m:128 * (m + 1),
                                   512 * n:512 * (n + 1)],
                        osb[:, 512 * n:512 * (n + 1)])

    nc.compile()
    return nc


_NC_CACHE = None
LAST_RESULT = None


def _get_nc():
    global _NC_CACHE
    if _NC_CACHE is None:
        _NC_CACHE = _build()
    return _NC_CACHE


def kernel(x, wq, wk, wv, wo, freqs_cos, freqs_sin, mask, start_pos=0):
    assert int(start_pos) == 0, "kernel specialized for start_pos == 0"
    import ml_dtypes
    x = np.asarray(x, np.float32)
    b, s, d = x.shape
    assert (b, s, d) == (1, S, D)
    xT = np.ascontiguousarray(x[0].T).astype(ml_dtypes.bfloat16)
    # pre-tile: xT[sq2, kc] = contiguous (128, 1024) block -> 2KB DMA lines
    xTt = np.ascontiguousarray(
        xT.reshape(16, 128, 2, 1024).transpose(2, 0, 1, 3))
    wq = np.asarray(wq, np.float32)
    wk = np.asarray(wk, np.float32)
    wv = np.asarray(wv, np.float32)
    wot = np.ascontiguousarray(
        np.asarray(wo, np.float32).reshape(16, 128, D).transpose(1, 0, 2)
    ).astype(ml_dtypes.bfloat16)
    cos = np.asarray(freqs_cos, np.float32)   # (S, 32)
    sin = np.asarray(freqs_sin, np.float32)
    cos5 = np.ascontiguousarray(
        np.tile(cos, (1, 5)).reshape(16, 128, RCOLS // 2).transpose(1, 0, 2)
    ).astype(ml_dtypes.bfloat16)
    sin5 = np.ascontiguousarray(
        np.tile(sin, (1, 5)).reshape(16, 128, RCOLS // 2).transpose(1, 0, 2)
    ).astype(ml_dtypes.bfloat16)
    maskT01 = np.ascontiguousarray(
        (np.asarray(mask, np.float32)[:128, :128].T == 0.0)
    ).astype(ml_dtypes.bfloat16)

    in_maps = []
    for c in range(N_CORES):
        wqkv_c = np.concatenate(
            [wq[:, QCOLS * c:QCOLS * (c + 1)],
             wk[:, HD * c:HD * (c + 1)],
             wv[:, HD * c:HD * (c + 1)]], axis=1)   # (2048, 384)
        # [g][p][kc_in_g][col]
        wqkv_t = np.ascontiguousarray(
            wqkv_c.reshape(4, 4, 128, PCOLS).transpose(0, 2, 1, 3)
        ).astype(ml_dtypes.bfloat16)
        in_maps.append({
            "xT": xTt,
            "wqkv": wqkv_t,
            "wo": wot,
            "cos5": cos5,
            "sin5": sin5,
            "maskT01": maskT01,
        })

    nc = _get_nc()
    res = bass_utils.run_bass_kernel_spmd(
        nc, in_maps, core_ids=list(range(N_CORES)),
        trace=bool(os.environ.get("BASS_TRACE")))
    global LAST_RESULT
    LAST_RESULT = res
    rows = [res.results[c]["out"] for c in range(N_CORES)]
    return np.concatenate(rows, axis=0).reshape(1, S, D).astype(np.float32)



# revision 13
# speedup vs baseline: 1.2881x; 1.1403x over previous
"""GQA attention (S=2048, D=2048, 32 q-heads / 8 kv-heads, rope, causal) on 8
Trainium2 NeuronCores, tensor-parallel over heads (1 kv head + 4 q heads per
core), chunked AllToAll re-shard overlapped with compute, row-sharded output.

v4 on top of v3 (trace-driven):
 - cos/sin/mask load as single contiguous DMAs (the chunked mid-axis slices
   fragmented into 640B descriptors that jammed the rings and stalled the
   PE ~40us waiting for x chunks queued behind them).
 - warm-up collective staged first on the sync ring -> cores barrier-align
   ~16us in instead of ~66us.
 - BOTH head-pairs' attention tiles t=0..2 interleave into stage P (kT/v
   are shared by the pairs); only the two t=3 tiles remain after block 15,
   so a2a(0) fires ~8us after stage P and a2a(1) ~8us later, with pair-1
   t3 and stage-W i=0 overlapping the collectives.
 - a2a staging DMAs issue incrementally as each tile is normalized.

v3 design vs the original baseline (408us):
 - stage P computes projections UNTRANSPOSED (seq on partitions, qkv dims on
   the free axis, one fused (128,384) q|k|v psum tile per 128-seq block).
   Rope pairs are then adjacent on the free axis, so the rotation runs as
   full-width (128,160) stride-2 vector ops instead of 30 (32,512)-shaped
   ops per seq tile. v lands directly in PV-lhs orientation; q/k are
   transposed by the PE via identity matmuls.
 - stage P is DMA-floor-paced (x streams ~9MB over 3 rings), so the first
   head-pair's ATTENTION IS INTERLEAVED INTO stage P: attention q-tile t
   only needs seq blocks 0..4t+3, so it runs right after P block 4t+3,
   filling the PE/ACT/DVE idle time under the DMA wait and firing the first
   AllToAll ~80us earlier. This also keeps the PE continuously busy -- its
   clock is activity-gated (HAM throttles to 1.2GHz when idling).
 - scores matmuls for the two heads of a pair run CONCURRENTLY in the PE
   array via row tiling (head A in rows 0:63 / tile_position (0,0), head B
   in rows 64:127 / (64,0)); kT is duplicated into partitions 64:128.
 - exp: diagonal tiles use exact ACT-table exp; full-width tiles are
   balanced between ACT and a paired-Schraudolph exp2 bit trick on the DVE
   (two f32->int16 tensor_scalar ops bitcast to bf16; the two PV matmuls
   against v and v/sqrt2 average the pair, cancelling most of the sawtooth
   error inside the softmax). Sub-B PV matmuls lag one block-step so the
   DVE latency never stalls the in-order PE.
 - PV matmuls use the full 128-col v tile (cols 65:128 zero) so fast weight
   load kicks in; the ones-column at col 64 yields softmax denominators.
 - stage W: i=1 pass is m-outer so the m=0 psum copy + output DMA overlap
   the m=1 matmuls.

Self-contained: takes full inputs, shards on host, runs one SPMD Bass/Tile
kernel via run_bass_kernel_spmd, reassembles the full output.
"""
import os
import sys
from contextlib import ExitStack

import numpy as np

try:
    import concourse.bass as bass  # noqa: F401
except ImportError:  # platform tree not on sys.path in a fresh dir
    sys.path.insert(0, "/opt/trn_rl_repo")
    import concourse.bass as bass  # noqa: F401

import concourse.mybir as mybir
from concourse import bacc, bass_utils, tile
from concourse.masks import make_identity

F32 = mybir.dt.float32
BF16 = mybir.dt.bfloat16
I16 = mybir.dt.int16
AF = mybir.ActivationFunctionType

S = 2048          # sequence length
D = 2048          # model dim
HD = 64           # head dim
N_CORES = 8
QH_PER_CORE = 4   # q heads per core (32/8)
QCOLS = QH_PER_CORE * HD       # 256 q-projection cols per core
PCOLS = QCOLS + 2 * HD         # 384 fused q|k|v cols per core
RCOLS = QCOLS + HD             # 320 roped cols (q|k)
ROWS_PER_CORE = S // N_CORES   # 256 output rows per core

SCH_A = 0.125 * 128 * 1.4426950408889634
# bias: 127*128 maps the exponent; -128 halves (the pair S1+S2/sqrt2 would
# otherwise sum to ~2*exp, mismatching exact-exp blocks in the same softmax
# chain); -6.87 removes the pair's +3.85% mean sawtooth bias so approx and
# exact blocks carry equal weight in the denominator.
SCH_B1 = 127.0 * 128.0 - 128.0 - 6.87
SCH_B2 = SCH_B1 + 64.0


def _build():
    nc = bacc.Bacc("TRN2", target_bir_lowering=False, debug=False,
                   num_devices=N_CORES)
    xT_d = nc.dram_tensor("xT", [16, 128, 16, 128], BF16,
                          kind="ExternalInput")
    wqkv_d = nc.dram_tensor("wqkv", [4, 128, 4, PCOLS], BF16,
                            kind="ExternalInput")
    wo_d = nc.dram_tensor("wo", [128, 16, D], BF16, kind="ExternalInput")
    cos5_d = nc.dram_tensor("cos5", [128, 16, RCOLS // 2], BF16,
                            kind="ExternalInput")
    sin5_d = nc.dram_tensor("sin5", [128, 16, RCOLS // 2], BF16,
                            kind="ExternalInput")
    mask_d = nc.dram_tensor("maskT01", [128, 128], BF16, kind="ExternalInput")
    out_d = nc.dram_tensor("out", [ROWS_PER_CORE, D], F32, kind="ExternalOutput")

    with tile.TileContext(nc) as tc, ExitStack() as top:
        persist = top.enter_context(tc.tile_pool(name="persist", bufs=1))
        # head-pair q tiles: rows = [head(2p) dims | head(2p+1) dims]
        qPs = [persist.tile([128, S], BF16, name=f"qP{i}", uniquify=False)
               for i in range(2)]
        # k dims on rows 0:64, duplicated to 64:128 for the paired scores mm
        kT = persist.tile([128, S], BF16, name="kT")
        v128 = persist.tile([128, 16, 128], BF16, name="v128")
        # v/sqrt2: second PV operand of the paired-Schraudolph approx path
        v128b = persist.tile([128, 16, 128], BF16, name="v128b")
        attnT0 = persist.tile([128, S], BF16, name="attnT0")
        attnT1 = persist.tile([128, S], BF16, name="attnT1")
        attnTs = [attnT0, attnT1]
        maskT_sb = persist.tile([128, 128], BF16, name="maskT_sb")
        cos5_sb = persist.tile([128, 16, RCOLS // 2], F32, name="cos5_sb")
        sin5_sb = persist.tile([128, 16, RCOLS // 2], F32, name="sin5_sb")
        wqkv_sb = persist.tile([128, 16, PCOLS], BF16, name="wqkv_sb")
        wo_sb = persist.tile([128, 16, D], BF16, name="wo_sb")
        identity = persist.tile([128, 128], F32, name="identity")
        wsmall = persist.tile([8, 64], BF16, name="wsmall")
        probs_pool = top.enter_context(tc.tile_pool(name="probs", bufs=3))
        nrm_pool = top.enter_context(tc.tile_pool(name="nrm", bufs=4))

        dram = top.enter_context(tc.tile_pool(name="dram", bufs=1, space="DRAM"))
        a2a_in = [dram.tile([N_CORES, 128, ROWS_PER_CORE], BF16,
                            name=f"a2a_in{i}", uniquify=False)
                  for i in range(2)]
        a2a_out = [dram.tile([N_CORES, 128, ROWS_PER_CORE], BF16,
                             name=f"a2a_out{i}", uniquify=False)
                   for i in range(2)]
        dum_in = dram.tile([N_CORES, 64], BF16, name="dum_in", uniquify=False)
        dum_out = dram.tile([N_CORES, 64], BF16, name="dum_out",
                            uniquify=False)

        # warm-up collective FIRST: its 1KB staging lands before the big
        # streams jam the rings, so the cc trigger (which barriers all 8
        # cores) fires ~16us in, aligning cores for the later a2a's.
        nc.vector.memset(wsmall[:], 0.0)
        nc.sync.dma_start(dum_in[:], wsmall[:])
        nc.gpsimd.collective_compute(
            "AllToAll", mybir.AluOpType.bypass,
            replica_groups=[list(range(N_CORES))],
            ins=[dum_in[:]], outs=[dum_out[:]])

        # wqkv chunks spread over the 3 DMA-capable rings
        nc.sync.dma_start(wqkv_sb[:, 0:4, :], wqkv_d.ap()[0])
        nc.scalar.dma_start(wqkv_sb[:, 4:8, :], wqkv_d.ap()[1])
        nc.gpsimd.dma_start(wqkv_sb[:, 8:12, :], wqkv_d.ap()[2])
        make_identity(nc, identity[:])

        dmaq = [nc.sync, nc.scalar, nc.gpsimd]

        def flat8(t, lo, hi):
            # contiguous half-table slice: flatten (blk, f) so the DMA is
            # one run per partition, not per (partition, blk)
            return t.rearrange("p b f -> p (b f)")[:, lo * (RCOLS // 2):
                                                   hi * (RCOLS // 2)]

        def startup_dmas(blk):
            # interleaved behind the first x tiles so neither delays the
            # other; cos/sin stay on gpsimd (SWDGE casts bf16->f32)
            if blk == 1:
                nc.scalar.dma_start(wqkv_sb[:, 12:16, :], wqkv_d.ap()[3])
            elif blk == 2:
                nc.gpsimd.dma_start(flat8(cos5_sb[:], 0, 8),
                                    flat8(cos5_d.ap(), 0, 8))
                nc.gpsimd.dma_start(flat8(sin5_sb[:], 0, 8),
                                    flat8(sin5_d.ap(), 0, 8))
            elif blk == 4:
                nc.scalar.dma_start(maskT_sb[:], mask_d.ap())
            elif blk == 6:
                nc.gpsimd.dma_start(flat8(cos5_sb[:], 8, 16),
                                    flat8(cos5_d.ap(), 8, 16))
                nc.gpsimd.dma_start(flat8(sin5_sb[:], 8, 16),
                                    flat8(sin5_d.ap(), 8, 16))

        # ---------------- attention machinery (used in both phases) --------
        # gpsimd cannot touch PSUM, so the exp2 bit-trick (reads scores psum)
        # only runs on the DVE.
        eng_load = {"act": 11000.0, "dve": 21000.0}

        def exp_cost(w):
            return {"act": 220 + 0.93 * w,
                    "dve": 2 * (170 + 1.0 * w)}

        def emit_probs(key, psc, po, col0, b, nb, diag, may_offload):
            """Emit the probs computation now; return a closure that emits
            the PV matmul(s) -- deferring it one block-step for sub B hides
            the DVE bit-trick's 2-op latency from the in-order PE."""
            w = 512 - col0
            if diag or not may_offload:
                eng = "act"
            else:
                cost = exp_cost(w)
                eng = min(cost, key=lambda e: eng_load[e] + cost[e])
            eng_load[eng] += exp_cost(w)[eng]
            if eng == "act":
                probs = probs_pool.tile([128, 512], BF16,
                                        name=f"pr{key}", tag="probs")
                nc.scalar.activation(probs[:, col0:512], psc[:, col0:512],
                                     AF.Exp, scale=0.125)
                if diag:
                    nc.vector.tensor_mul(probs[:, col0:col0 + 128],
                                         probs[:, col0:col0 + 128],
                                         maskT_sb[:])

                def pv():
                    nc.tensor.matmul(po[:, col0:512], v128[:, b, :],
                                     probs[:, col0:512],
                                     start=(b == 0), stop=(b == nb - 1))
            else:
                i1 = probs_pool.tile([128, 512], I16, name=f"i1{key}",
                                     tag="i1")
                i2 = probs_pool.tile([128, 512], I16, name=f"i2{key}",
                                     tag="i2")
                nc.vector.tensor_scalar(
                    i1[:, col0:512], psc[:, col0:512], SCH_A, SCH_B1,
                    mybir.AluOpType.mult, mybir.AluOpType.add)
                nc.vector.tensor_scalar(
                    i2[:, col0:512], psc[:, col0:512], SCH_A, SCH_B2,
                    mybir.AluOpType.mult, mybir.AluOpType.add)

                def pv():
                    nc.tensor.matmul(po[:, col0:512], v128[:, b, :],
                                     i1[:, col0:512].bitcast(BF16),
                                     start=(b == 0), stop=False)
                    nc.tensor.matmul(po[:, col0:512], v128b[:, b, :],
                                     i2[:, col0:512].bitcast(BF16),
                                     start=False, stop=(b == nb - 1))
            return pv

        def stage_a2a(i, t):
            # stage tile t's two 256-seq rows into DRAM as soon as the tile
            # is normalized, so only rows 6,7 + the trigger remain at the end
            for r in (2 * t, 2 * t + 1):
                nc.sync.dma_start(a2a_in[i][r],
                                  attnTs[i][:, 256 * r:256 * (r + 1)])

        def send_a2a(i):
            nc.gpsimd.collective_compute(
                "AllToAll", mybir.AluOpType.bypass,
                replica_groups=[list(range(N_CORES))],
                ins=[a2a_in[i][:]], outs=[a2a_out[i][:]])

        def finish(h, t, po):
            # den bounces through SBUF: reciprocal_approx_fast is a raw
            # bit-trick (BITWISE_NOT seed) and must read SBUF bits
            den = nrm_pool.tile([1, 512], F32, name=f"dn{h}{t}", tag="den")
            nc.scalar.copy(den[:], po[HD:HD + 1, :])
            recip = nrm_pool.tile([1, 512], F32, name=f"rc{h}{t}",
                                  tag="recip")
            nc.vector.reciprocal_approx_fast(recip[:], den[:])
            rfac = nrm_pool.tile([HD, 512], F32, name=f"rf{h}{t}", tag="rfac")
            nc.gpsimd.partition_broadcast(rfac[:], recip[:])
            nc.vector.tensor_mul(
                attnTs[h // 2][64 * (h % 2):64 * (h % 2) + HD,
                               512 * t:512 * (t + 1)],
                po[0:HD, :], rfac[:])

        def attention_steps(p, t, pscA_pool, pscB_pool, po_pool,
                            runahead):
            """Generator: yields after each block-step so the caller can
            interleave projection work between steps (the PE queue is FIFO;
            a whole tile emitted at once stalls it on every exp)."""
            qP = qPs[p]
            nb = 4 * t + 4
            poA = po_pool.tile([128, 512], F32, name=f"poA{p}{t}", tag="poA")
            poB = po_pool.tile([128, 512], F32, name=f"poB{p}{t}", tag="poB")
            pscs = {}

            def scores(b):
                j = max(0, b - 4 * t)
                col0 = 128 * j
                pscA = pscA_pool.tile([128, 512], F32,
                                      name=f"pA{p}{t}{b}", tag="pscA")
                pscB = pscB_pool.tile([128, 512], F32,
                                      name=f"pB{p}{t}{b}", tag="pscB")
                qc = (512 * t + col0, 512 * (t + 1))
                kc_ = (128 * b, 128 * (b + 1))
                # concurrent in the PE: rows 0:63 and 64:127
                nc.tensor.matmul(pscA[:, col0:512], kT[0:64, kc_[0]:kc_[1]],
                                 qP[0:64, qc[0]:qc[1]], start=True, stop=True)
                nc.tensor.matmul(pscB[:, col0:512], kT[64:128, kc_[0]:kc_[1]],
                                 qP[64:128, qc[0]:qc[1]],
                                 start=True, stop=True)
                pscs[b] = (pscA, pscB, col0)

            for b in range(min(runahead + 1, nb)):
                scores(b)
            pend_b = None
            for b in range(nb):
                pscA, pscB, col0 = pscs.pop(b)
                diag = b >= 4 * t
                if pend_b is not None:
                    pend_b()
                pvA = emit_probs(f"{p}{t}{b}0", pscA, poA, col0,
                                 b, nb, diag, may_offload=False)
                pvA()
                pend_b = emit_probs(f"{p}{t}{b}1", pscB, poB, col0,
                                    b, nb, diag, may_offload=True)
                # scores for the lookahead step go LAST: the PE queue is
                # in-order, so a WAR-stalled scores mm must not block the
                # ready PV/probs work above
                if b + runahead + 1 < nb:
                    scores(b + runahead + 1)
                yield
            pend_b()
            finish(2 * p, t, poA)
            finish(2 * p + 1, t, poB)
            stage_a2a(p, t)

        def attention_tile(p, t, pscA_pool, pscB_pool, po_pool,
                           runahead):
            for _ in attention_steps(p, t, pscA_pool, pscB_pool, po_pool,
                                     runahead):
                pass

        # ------- Stage P (+ BOTH pairs' attention t=0..2 interleaved) ------
        with ExitStack() as ctx:
            xtb_pool = ctx.enter_context(tc.tile_pool(name="xtb", bufs=6))
            S_pool = ctx.enter_context(
                tc.tile_pool(name="Sp", bufs=2, space="PSUM"))
            pvt_pool = ctx.enter_context(
                tc.tile_pool(name="pvt", bufs=1, space="PSUM"))
            tmp_pool = ctx.enter_context(tc.tile_pool(name="ropetmp", bufs=2))
            pscIA_pool = ctx.enter_context(
                tc.tile_pool(name="pscIA", bufs=2, space="PSUM"))
            pscIB_pool = ctx.enter_context(
                tc.tile_pool(name="pscIB", bufs=1, space="PSUM"))
            poI_pool = ctx.enter_context(
                tc.tile_pool(name="poI", bufs=1, space="PSUM"))

            nc.vector.memset(v128[:, :, HD:HD + 1], 1.0)
            nc.vector.memset(v128[:, :, HD + 1:], 0.0)
            nc.vector.memset(v128b[:, :, HD:HD + 1], 0.70710678)
            nc.vector.memset(v128b[:, :, HD + 1:], 0.0)

            def evens(ap):
                return ap.rearrange("p (n two) -> p two n", two=2)[:, 0, :]

            def odds(ap):
                return ap.rearrange("p (n two) -> p two n", two=2)[:, 1, :]

            def rope_block(blk, Sp):
                cosb = cos5_sb[:, blk, :]
                sinb = sin5_sb[:, blk, :]
                Se = evens(Sp[:, 0:RCOLS])
                So = odds(Sp[:, 0:RCOLS])
                t1 = tmp_pool.tile([128, RCOLS // 2], F32, name=f"t1_{blk}",
                                   tag="t1")
                t2 = tmp_pool.tile([128, RCOLS // 2], F32, name=f"t2_{blk}",
                                   tag="t2")
                t3 = tmp_pool.tile([128, RCOLS // 2], F32, name=f"t3_{blk}",
                                   tag="t3")
                t4 = tmp_pool.tile([128, RCOLS // 2], F32, name=f"t4_{blk}",
                                   tag="t4")
                stg = tmp_pool.tile([128, RCOLS], F32, name=f"stg_{blk}",
                                    tag="stg")
                nc.vector.tensor_mul(t1[:], Se, cosb)
                nc.vector.tensor_mul(t2[:], So, sinb)
                nc.vector.tensor_mul(t3[:], Se, sinb)
                nc.vector.tensor_mul(t4[:], So, cosb)
                nc.gpsimd.tensor_sub(evens(stg[:]), t1[:], t2[:])
                nc.gpsimd.tensor_add(odds(stg[:]), t3[:], t4[:])
                # v drops straight into PV-lhs orientation
                nc.scalar.copy(v128[:, blk, 0:HD], Sp[:, RCOLS:PCOLS])
                nc.scalar.mul(v128b[:, blk, 0:HD], Sp[:, RCOLS:PCOLS],
                              0.70710678)
                # transpose roped q (2x128 dims) and k (64) into column-major
                for g in range(2):
                    pvq = pvt_pool.tile([128, 128], F32, name=f"pvq{blk}{g}",
                                        tag="pvq")
                    nc.tensor.transpose(pvq[:], stg[:, 128 * g:128 * (g + 1)],
                                        identity[:])
                    nc.scalar.copy(qPs[g][:, 128 * blk:128 * (blk + 1)],
                                   pvq[:])
                # k transpose shares the pvq tag/bank (rows 0:64) so pvt
                # stays a single PSUM bank
                pvk = pvt_pool.tile([128, 128], F32, name=f"pvk{blk}",
                                    tag="pvq")
                nc.tensor.transpose(pvk[0:HD, :], stg[:, QCOLS:RCOLS],
                                    identity[:])
                nc.scalar.copy(kT[0:HD, 128 * blk:128 * (blk + 1)],
                               pvk[0:HD, :])
                nc.scalar.copy(kT[HD:128, 128 * blk:128 * (blk + 1)],
                               pvk[0:HD, :])

            # BOTH pairs' attention tiles t=0..2 ride the DMA slack of stage
            # P (kT/v are shared by the two pairs; tile t only needs seq
            # blocks 0..4t+3). Sequential generators keep PSUM at 7 banks:
            # S 2 + pvt 1 + pscA/B 2 + poA/B 2.
            gens = []

            def pump(n):
                done = 0
                while gens and done < n:
                    try:
                        next(gens[0])
                        done += 1
                    except StopIteration:
                        gens.pop(0)
                return done

            xtbs = []
            for blk in range(16):
                # one contiguous 512KB DMA per block: all 16 dim-chunks for
                # this block's 128 seq columns, so block 0 completes ~25us
                # in instead of waiting for the whole half of x
                xtb = xtb_pool.tile([128, 16, 128], BF16,
                                    name=f"xtb{blk}", tag="xtb")
                dmaq[blk % 3].dma_start(xtb[:], xT_d.ap()[blk])
                xtbs.append(xtb)
                startup_dmas(blk)
            for blk in range(16):
                Sp = S_pool.tile([128, PCOLS], F32, name=f"S{blk}", tag="S")
                for kc in range(16):
                    nc.tensor.matmul(
                        Sp[:], xtbs[blk][:, kc, :],
                        wqkv_sb[:, kc, :],
                        start=(kc == 0), stop=(kc == 15))
                    if kc % 3 == 2:
                        pump(1)
                rope_block(blk, Sp)
                if blk % 4 == 3 and blk < 15:
                    gens.append(attention_steps(
                        0, blk // 4, pscIA_pool, pscIB_pool, poI_pool,
                        runahead=1))
                    gens.append(attention_steps(
                        1, blk // 4, pscIA_pool, pscIB_pool, poI_pool,
                        runahead=1))
            while pump(1):
                pass

            # wo prefetch for stage W; off the sync ring so the a2a_in
            # staging DMAs aren't queued behind it
            nc.scalar.dma_start(wo_sb[:, 0:8, :], wo_d.ap()[:, 0:8, :])
            nc.gpsimd.dma_start(wo_sb[:, 8:16, :], wo_d.ap()[:, 8:16, :])

        # --------- Tail: the two t=3 tiles (need proj block 15) + A2As ----
        with ExitStack() as ctx:
            pscA_pool = ctx.enter_context(
                tc.tile_pool(name="pscA", bufs=2, space="PSUM"))
            pscB_pool = ctx.enter_context(
                tc.tile_pool(name="pscB", bufs=2, space="PSUM"))
            po_pool = ctx.enter_context(
                tc.tile_pool(name="po", bufs=2, space="PSUM"))
            attention_tile(0, 3, pscA_pool, pscB_pool, po_pool, runahead=1)
            send_a2a(0)          # cc runs while pair-1 t3 computes
            attention_tile(1, 3, pscA_pool, pscB_pool, po_pool, runahead=1)
            send_a2a(1)

        # Stage W: out rows = attn_fullT.T @ wo, accumulated in two passes
        # (even h-chunks from a2a chunk 0, odd from chunk 1). The i=1 pass is
        # m-outer so m=0's psum copy + out DMA overlap m=1's matmuls.
        with ExitStack() as ctx:
            af_pool = ctx.enter_context(tc.tile_pool(name="af", bufs=1))
            pw_pool = ctx.enter_context(
                tc.tile_pool(name="pw", bufs=1, space="PSUM"))
            osb_pool = ctx.enter_context(tc.tile_pool(name="osb", bufs=2))
            afs = []
            for i, eng in ((0, nc.sync), (1, nc.scalar)):
                af = af_pool.tile([128, N_CORES, ROWS_PER_CORE], BF16,
                                  name=f"attn_full{i}", uniquify=False)
                eng.dma_start(af[:],
                              a2a_out[i][:].rearrange("r p s -> p r s"))
                afs.append(af)
            pw = [[pw_pool.tile([128, 512], F32, name=f"pw{m}{n}",
                                tag=f"pw{m}{n}") for n in range(4)]
                  for m in range(2)]
            for r in range(N_CORES):          # i = 0: even h-chunks
                for m in range(2):
                    lhs = afs[0][:, r, 128 * m:128 * (m + 1)]
                    for n in range(4):
                        nc.tensor.matmul(
                            pw[m][n][:], lhs,
                            wo_sb[:, 2 * r, 512 * n:512 * (n + 1)],
                            start=(r == 0), stop=False)
            for m in range(2):                # i = 1: odd h-chunks, m-outer
                for r in range(N_CORES):
                    lhs = afs[1][:, r, 128 * m:128 * (m + 1)]
                    for n in range(4):
                        nc.tensor.matmul(
                            pw[m][n][:], lhs,
                            wo_sb[:, 2 * r + 1, 512 * n:512 * (n + 1)],
                            start=False, stop=(r == N_CORES - 1))
                osb = osb_pool.tile([128, D], F32, name=f"osb{m}", tag="osb")
                for n in range(4):
                    nc.scalar.copy(osb[:, 512 * n:512 * (n + 1)], pw[m][n][:])
                    eng = nc.sync if n % 2 == 0 else nc.scalar
                    eng.dma_start(
                        out_d.ap()[128 * m:128 * (m + 1),
                                   512 * n:512 * (n + 1)],
                        osb[:, 512 * n:512 * (n + 1)])

    nc.compile()
    return nc


_NC_CACHE = None
LAST_RESULT = None


def _get_nc():
    global _NC_CACHE
    if _NC_CACHE is None:
        _NC_CACHE = _build()
    return _NC_CACHE


def kernel(x, wq, wk, wv, wo, freqs_cos, freqs_sin, mask, start_pos=0):
    assert int(start_pos) == 0, "kernel specialized for start_pos == 0"
    import ml_dtypes
    x = np.asarray(x, np.float32)
    b, s, d = x.shape
    assert (b, s, d) == (1, S, D)
    xT = np.ascontiguousarray(x[0].T).astype(ml_dtypes.bfloat16)
    # pre-tile: xTt[j][p][kc][s] -> per seq-block 512KB contiguous, 4KB
    # per partition line
    xTt = np.ascontiguousarray(
        xT.reshape(16, 128, 16, 128).transpose(2, 1, 0, 3))
    wq = np.asarray(wq, np.float32)
    wk = np.asarray(wk, np.float32)
    wv = np.asarray(wv, np.float32)
    wot = np.ascontiguousarray(
        np.asarray(wo, np.float32).reshape(16, 128, D).transpose(1, 0, 2)
    ).astype(ml_dtypes.bfloat16)
    cos = np.asarray(freqs_cos, np.float32)   # (S, 32)
    sin = np.asarray(freqs_sin, np.float32)
    cos5 = np.ascontiguousarray(
        np.tile(cos, (1, 5)).reshape(16, 128, RCOLS // 2).transpose(1, 0, 2)
    ).astype(ml_dtypes.bfloat16)
    sin5 = np.ascontiguousarray(
        np.tile(sin, (1, 5)).reshape(16, 128, RCOLS // 2).transpose(1, 0, 2)
    ).astype(ml_dtypes.bfloat16)
    maskT01 = np.ascontiguousarray(
        (np.asarray(mask, np.float32)[:128, :128].T == 0.0)
    ).astype(ml_dtypes.bfloat16)

    in_maps = []
    for c in range(N_CORES):
        wqkv_c = np.concatenate(
            [wq[:, QCOLS * c:QCOLS * (c + 1)],
             wk[:, HD * c:HD * (c + 1)],
             wv[:, HD * c:HD * (c + 1)]], axis=1)   # (2048, 384)
        # [g][p][kc_in_g][col]
        wqkv_t = np.ascontiguousarray(
            wqkv_c.reshape(4, 4, 128, PCOLS).transpose(0, 2, 1, 3)
        ).astype(ml_dtypes.bfloat16)
        in_maps.append({
            "xT": xTt,
            "wqkv": wqkv_t,
            "wo": wot,
            "cos5": cos5,
            "sin5": sin5,
            "maskT01": maskT01,
        })

    nc = _get_nc()
    res = bass_utils.run_bass_kernel_spmd(
        nc, in_maps, core_ids=list(range(N_CORES)),
        trace=bool(os.environ.get("BASS_TRACE")))
    global LAST_RESULT
    LAST_RESULT = res
    rows = [res.results[c]["out"] for c in range(N_CORES)]
    return np.concatenate(rows, axis=0).reshape(1, S, D).astype(np.float32)

